# revision 1
# baseline (speedup 1.0000x reference)
"""MultiHeadAttention TRN2 kernel: tensor-parallel over heads across 8 NeuronCores.

Problem (hardcoded): BS=2, QLEN=2048, DIM=1024, NHEADS=16, HEAD=64.
  q = split_heads(x @ q_w.T + q_b) / sqrt(64)
  s = q @ k.T + mask ; w = softmax(s) ; ctx = w @ v
  out = merge_heads(ctx) @ o_w.T + o_b

Sharding: core c computes heads {2c, 2c+1} (rows 128c:128c+128 of q/k/v weights,
cols 128c:128c+128 of o_w).  Each core emits a full-shape partial of the output
projection; the host sums the 8 partials and adds o_b (row-parallel gather).

Device layout (all "T" tensors are [feature, token] so the contraction dim of
every matmul sits on SBUF partitions):
  QT/KT/VT [128, 4096]  = projections for 2 local heads (partitions = head dims)
  scores^T tiles [128 kpos, q] via row-tiled matmul pairs (one head per 64-row
  group of the PE array); exp() on ScalarE with the attention mask fed through
  the activation bias port (mask varies along k = partitions).
  PV uses col-tiled pairs: ctx^T lands as [128 local dims, q] = exactly the
  layout the output projection needs.  Softmax denominators come from matmuls
  against an all-ones [128,64] stationary operand, which broadcasts each head's
  sum across that head's 64 psum partitions -> reciprocal+multiply on VectorE
  with no cross-partition moves.
"""

import sys

if "/opt/trn_rl_repo" not in sys.path:
    sys.path.insert(0, "/opt/trn_rl_repo")

import math
from contextlib import ExitStack

import ml_dtypes
import numpy as np

import concourse.bass as bass
import concourse.tile as tile
from concourse import bacc, mybir
from concourse.bass_utils import run_bass_kernel_spmd


# ---- problem constants ----
BS, QLEN, DIM, NHEADS = 2, 2048, 1024, 16
HEAD = DIM // NHEADS            # 64
NTOK = BS * QLEN                # 4096
NCORES = 8
HPC = NHEADS // NCORES          # 2 heads per core
LDIM = HPC * HEAD               # 128 local dims per core
NKCH = DIM // 128               # 8 contraction chunks for projections
NTT = NTOK // 512               # 8 token tiles of 512 for projections
NKT = QLEN // 128               # 16 key tiles per batch
QTW = 1024                      # query tile width for attention
NQT = QLEN // QTW               # 2 query tiles per batch

DT = mybir.dt.bfloat16          # matmul compute dtype
NPDT = ml_dtypes.bfloat16
F32 = mybir.dt.float32

_cache = {}


def build_program():
    """Build + compile the single-core SPMD Bass program."""
    nc = bacc.Bacc("TRN2", target_bir_lowering=False, debug=False,
                   num_devices=NCORES)

    xt = nc.dram_tensor("xt", [DIM, NTOK], DT, kind="ExternalInput").ap()
    wq = nc.dram_tensor("wq", [DIM, LDIM], DT, kind="ExternalInput").ap()
    wk = nc.dram_tensor("wk", [DIM, LDIM], DT, kind="ExternalInput").ap()
    wv = nc.dram_tensor("wv", [DIM, LDIM], DT, kind="ExternalInput").ap()
    wo = nc.dram_tensor("wo", [LDIM, DIM], DT, kind="ExternalInput").ap()
    qb = nc.dram_tensor("qb", [LDIM, 1], F32, kind="ExternalInput").ap()
    kb = nc.dram_tensor("kb", [LDIM, 1], F32, kind="ExternalInput").ap()
    vb = nc.dram_tensor("vb", [LDIM, 1], F32, kind="ExternalInput").ap()
    maskd = nc.dram_tensor("maskd", [128, BS * NKT], F32,
                           kind="ExternalInput").ap()
    out = nc.dram_tensor("out", [NTOK, DIM], F32, kind="ExternalOutput").ap()

    with tile.TileContext(nc) as tc, ExitStack() as ctx:
        singles = ctx.enter_context(tc.tile_pool(name="singles", bufs=1))
        xpool = ctx.enter_context(tc.tile_pool(name="xs", bufs=3))
        evict = ctx.enter_context(tc.tile_pool(name="evict", bufs=2))
        work = ctx.enter_context(
            tc.tile_pool(name="work", bufs=3, space="PSUM"))
        accp = ctx.enter_context(
            tc.tile_pool(name="accp", bufs=1, space="PSUM"))

        # --- resident SBUF tensors ---
        wq_sb = singles.tile([128, NKCH, LDIM], DT, tag="wq")
        wk_sb = singles.tile([128, NKCH, LDIM], DT, tag="wk")
        wv_sb = singles.tile([128, NKCH, LDIM], DT, tag="wv")
        wo_sb = singles.tile([LDIM, DIM], DT, tag="wo")
        qb_sb = singles.tile([LDIM, 1], F32, tag="qb")
        kb_sb = singles.tile([LDIM, 1], F32, tag="kb")
        vb_sb = singles.tile([LDIM, 1], F32, tag="vb")
        mask_sb = singles.tile([128, BS * NKT], F32, tag="mask")
        ones_sb = singles.tile([128, HEAD], DT, tag="ones")
        qt_sb = singles.tile([128, NTOK], DT, tag="qt")
        kt_sb = singles.tile([128, NTOK], DT, tag="kt")
        vt_sb = singles.tile([128, NTOK], DT, tag="vt")
        ct_sb = singles.tile([128, NTOK], DT, tag="ct")
        v_sb = singles.tile([128, BS, NKT * HPC * HEAD], DT, tag="v")
        st_sb = singles.tile([128, NKT * HPC, QTW], DT, tag="st")
        recip_p = ctx.enter_context(tc.tile_pool(name="recip", bufs=2))

        for w_sb, w_dram in ((wq_sb, wq), (wk_sb, wk), (wv_sb, wv)):
            nc.sync.dma_start(
                w_sb[:], w_dram.rearrange("(c p) m -> p c m", p=128))
        nc.sync.dma_start(wo_sb[:], wo[:])
        nc.sync.dma_start(qb_sb[:], qb[:])
        nc.sync.dma_start(kb_sb[:], kb[:])
        nc.sync.dma_start(vb_sb[:], vb[:])
        nc.sync.dma_start(mask_sb[:], maskd[:])
        nc.vector.memset(ones_sb[:], 1.0)

        # --- phase 1: QKV projections, token-tile at a time ---
        # QT/KT/VT[p, t] for p in local head dims; batches in token order so
        # batch 0 attention can start while batch 1 projections still run.
        for g in range(NTT):
            psqk = work.tile([128, 1024], F32, tag="work")
            psv = work.tile([128, 1024], F32, tag="work")
            for c in range(NKCH):
                xt_t = xpool.tile([128, 512], DT, tag="xt")
                nc.sync.dma_start(
                    xt_t[:], xt[128 * c:128 * (c + 1), 512 * g:512 * (g + 1)])
                st, sp = (c == 0), (c == NKCH - 1)
                nc.tensor.matmul(psqk[:, 0:512], wq_sb[:, c, :], xt_t[:],
                                 start=st, stop=sp)
                nc.tensor.matmul(psqk[:, 512:1024], wk_sb[:, c, :], xt_t[:],
                                 start=st, stop=sp)
                nc.tensor.matmul(psv[:, 0:512], wv_sb[:, c, :], xt_t[:],
                                 start=st, stop=sp)
            gs = slice(512 * g, 512 * (g + 1))
            nc.vector.tensor_scalar_add(qt_sb[:, gs], psqk[:, 0:512],
                                        qb_sb[:, 0:1])
            nc.vector.tensor_scalar_add(kt_sb[:, gs], psqk[:, 512:1024],
                                        kb_sb[:, 0:1])
            nc.vector.tensor_scalar_add(vt_sb[:, gs], psv[:, 0:512],
                                        vb_sb[:, 0:1])

            # after this batch's VT is done, transpose it to V tiles via DMA
            # transpose: V[b][128 kpos, 64] per (kt, h) = PV stationary operand.
            # (PE transpose-mode with bf16 psum output crashes TRN2 here.)
            if g in (NTT // 2 - 1, NTT - 1):
                b = 0 if g == NTT // 2 - 1 else 1
                for idx in range(2 * NKT):     # idx = kt*2 + h
                    kt_i, h = idx // 2, idx % 2
                    src = vt_sb[HEAD * h:HEAD * (h + 1),
                                QLEN * b + 128 * kt_i:
                                QLEN * b + 128 * (kt_i + 1)]
                    nc.sync.dma_start(
                        v_sb[:, b, 64 * idx:64 * (idx + 1)], src,
                        transpose=True)

        # --- phase 2+3: attention + output projection per (batch, qtile) ---
        for b in range(BS):
            for qt_i in range(NQT):
                qs = slice(QLEN * b + QTW * qt_i, QLEN * b + QTW * (qt_i + 1))
                ct_ps = accp.tile([128, QTW], F32, tag="acc")
                for kt_i in range(NKT):
                    ks = slice(QLEN * b + 128 * kt_i,
                               QLEN * b + 128 * (kt_i + 1))
                    m_ap = mask_sb[:, b * NKT + kt_i:b * NKT + kt_i + 1]
                    sts = []
                    for h in range(2):
                        hs = slice(HEAD * h, HEAD * (h + 1))
                        s_ps = work.tile([128, QTW], F32, tag="work")
                        for j in range(QTW // 512):
                            qsub = slice(qs.start + 512 * j,
                                         qs.start + 512 * (j + 1))
                            nc.tensor.matmul(
                                s_ps[:, 512 * j:512 * (j + 1)],
                                kt_sb[hs, ks], qt_sb[hs, qsub],
                                start=True, stop=True,
                                tile_position=(HEAD * h, 0))
                        st_t = st_sb[:, kt_i * 2 + h, :]
                        nc.scalar.activation(
                            st_t, s_ps[:], mybir.ActivationFunctionType.Exp,
                            bias=m_ap)
                        sts.append(st_t)
                    st0, sp0 = (kt_i == 0), (kt_i == NKT - 1)
                    for h in range(2):
                        vsl = v_sb[:, b, :].rearrange(
                            "p (i d) -> p i d", d=HEAD)[:, kt_i * 2 + h, :]
                        for j in range(QTW // 512):
                            nc.tensor.matmul(
                                ct_ps[HEAD * h:HEAD * (h + 1),
                                      512 * j:512 * (j + 1)],
                                vsl, sts[h][:, 512 * j:512 * (j + 1)],
                                start=st0, stop=sp0,
                                tile_position=(0, HEAD * h),
                                skip_group_check=True)

                # softmax denominators: ones.T @ exp(scores) broadcast across
                # each head's 64 partitions; then ct = ct * (1/sums).
                sum_ps = work.tile([128, QTW], F32, tag="work")
                for kt_i in range(NKT):
                    st0, sp0 = (kt_i == 0), (kt_i == NKT - 1)
                    for h in range(2):
                        for j in range(QTW // 512):
                            nc.tensor.matmul(
                                sum_ps[HEAD * h:HEAD * (h + 1),
                                       512 * j:512 * (j + 1)],
                                ones_sb[:], st_sb[:, kt_i * 2 + h,
                                                  512 * j:512 * (j + 1)],
                                start=st0, stop=sp0,
                                tile_position=(0, HEAD * h),
                                skip_group_check=True)
                rc = recip_p.tile([128, QTW], F32, tag="rc")
                nc.vector.reciprocal(rc[:], sum_ps[:])
                nc.vector.tensor_mul(ct_sb[:, qs], ct_ps[:], rc[:])

                # output projection for this qtile's tokens (one-shot matmuls)
                for t in range(QTW // 128):
                    tok0 = qs.start + 128 * t
                    o_ps = work.tile([128, 1024], F32, tag="work")
                    lhs = ct_sb[:, tok0:tok0 + 128]
                    nc.tensor.matmul(o_ps[:, 0:512], lhs, wo_sb[:, 0:512],
                                     start=True, stop=True)
                    nc.tensor.matmul(o_ps[:, 512:1024], lhs,
                                     wo_sb[:, 512:1024], start=True, stop=True)
                    o_sb = evict.tile([128, 1024], F32, tag="osb")
                    nc.vector.tensor_copy(o_sb[:], o_ps[:])
                    nc.sync.dma_start(out[tok0:tok0 + 128, :], o_sb[:])

    nc.compile()
    return nc


def shard_inputs(input, mask, q_w, q_b, k_w, k_b, v_w, v_b, o_w, o_b):
    x = np.asarray(input, np.float32)
    xt = np.ascontiguousarray(x.T).astype(NPDT)
    m = np.asarray(mask, np.float32).reshape(BS, NKT, 128)
    maskd = np.ascontiguousarray(m.transpose(2, 0, 1).reshape(128, BS * NKT))
    scale = 1.0 / math.sqrt(HEAD)
    in_maps = []
    for c in range(NCORES):
        L = slice(LDIM * c, LDIM * (c + 1))
        in_maps.append({
            "xt": xt,
            "wq": np.ascontiguousarray((q_w[L, :] * scale).T).astype(NPDT),
            "wk": np.ascontiguousarray(k_w[L, :].T).astype(NPDT),
            "wv": np.ascontiguousarray(v_w[L, :].T).astype(NPDT),
            "wo": np.ascontiguousarray(o_w[:, L].T).astype(NPDT),
            "qb": (q_b[L] * scale).astype(np.float32).reshape(LDIM, 1),
            "kb": k_b[L].astype(np.float32).reshape(LDIM, 1),
            "vb": v_b[L].astype(np.float32).reshape(LDIM, 1),
            "maskd": maskd,
        })
    return in_maps


def run(in_maps, **kw):
    if "nc" not in _cache:
        _cache["nc"] = build_program()
    return run_bass_kernel_spmd(_cache["nc"], in_maps,
                                core_ids=list(range(NCORES)), **kw)


def kernel(input, mask, q_w, q_b, k_w, k_b, v_w, v_b, o_w, o_b,
           bs=BS, qlen=QLEN):
    assert int(bs) == BS and int(qlen) == QLEN
    in_maps = shard_inputs(np.asarray(input), np.asarray(mask),
                           np.asarray(q_w), np.asarray(q_b),
                           np.asarray(k_w), np.asarray(k_b),
                           np.asarray(v_w), np.asarray(v_b),
                           np.asarray(o_w), np.asarray(o_b))
    res = run(in_maps)
    acc = np.zeros((NTOK, DIM), np.float32)
    for r in res.results:
        acc += r["out"]
    acc += np.asarray(o_b, np.float32)[None, :]
    return acc



# revision 19
# speedup vs baseline: 1.2456x; 1.2456x over previous
"""MultiHeadAttention TRN2 kernel: tensor-parallel over heads across 8 NeuronCores.

Problem (hardcoded): BS=2, QLEN=2048, DIM=1024, NHEADS=16, HEAD=64.
  q = split_heads(x @ q_w.T + q_b) / sqrt(64)
  s = q @ k.T + mask ; w = softmax(s) ; ctx = w @ v
  out = merge_heads(ctx) @ o_w.T + o_b

Sharding: core c computes heads {2c, 2c+1} (rows 128c:128c+128 of q/k/v weights,
cols 128c:128c+128 of o_w).  Each core emits a full-shape partial of the output
projection; the host sums the 8 partials and adds o_b (row-parallel gather).

Device layout (all "T" tensors are [feature, token] so the contraction dim of
every matmul sits on SBUF partitions):
  QT/KT/VT [128, 4096]  = projections for 2 local heads (partitions = head dims)
  scores^T tiles [128 kpos, q]; exp() on ScalarE with the attention mask fed
  through the activation bias port (mask varies along k = partitions).
  PV stationary operands are V tiles [128 kpos, 65] with an extra all-ones
  column, so each PV accumulation also produces the softmax denominators for
  free (they land on one extra PSUM partition).  Head 0 uses [V|1] -> psum
  partitions 0..64 (sums at 64); head 1 uses [1|V] -> psum partitions 63..127
  (sums at 63), which leaves both heads' context dims aligned with the
  [2*64, q] layout the output projection wants.
  Denominators: copy the two sums rows to SBUF (bf16), broadcast to all 128
  partitions with a tiny [2,128]-stationary matmul, reciprocal_approx_fast on
  DVE, then two elementwise multiplies produce normalized ctx^T in SBUF.

Scheduling notes:
  - The attention inner loop is software-pipelined: scores(kt+1) runs on the
    PE while ScalarE exps kt and PV(kt-1) trails, keeping the PE continuously
    busy (it only reaches its top p-state after ~3us of continuous work).
  - All x input loads issue from the Pool queue and V transposes from Sync,
    spread one group at a time, so neither starves the other.
  - PSUM budget (8 banks): work pool 2x[128,1024]f32 + acc pool 2x[128,1024]f32.
"""

import sys

if "/opt/trn_rl_repo" not in sys.path:
    sys.path.insert(0, "/opt/trn_rl_repo")

import math
from contextlib import ExitStack

import ml_dtypes
import numpy as np

import concourse.bass as bass
import concourse.tile as tile
from concourse import bacc, mybir
from concourse.bass_utils import run_bass_kernel_spmd


# ---- problem constants ----
BS, QLEN, DIM, NHEADS = 2, 2048, 1024, 16
HEAD = DIM // NHEADS            # 64
NTOK = BS * QLEN                # 4096
NCORES = 8
HPC = NHEADS // NCORES          # 2 heads per core
LDIM = HPC * HEAD               # 128 local dims per core
NKCH = DIM // 128               # 8 contraction chunks for projections
NTT = NTOK // 512               # 8 token tiles of 512 for projections
NKT = QLEN // 128               # 16 key tiles per batch
QTW = 1024                      # query tile width for attention
NQT = QLEN // QTW               # 2 query tiles per batch

DT = mybir.dt.bfloat16          # matmul compute dtype
NPDT = ml_dtypes.bfloat16
F32 = mybir.dt.float32

_cache = {}


def build_program(dump=False):
    """Build + compile the single-core SPMD Bass program."""
    nc = bacc.Bacc("TRN2", target_bir_lowering=False, debug=False,
                   num_devices=NCORES)
    dbg = {}
    if dump:
        for nm, shp, dt_ in (("d_qt", [128, NTOK], DT),
                             ("d_kt", [128, NTOK], DT),
                             ("d_vt", [128, NTOK], DT),
                             ("d_v65", [128, BS * 2 * NKT * 128], DT),
                             ("d_ct", [128, NTOK], DT),
                             ("d_rc", [128, QTW], F32)):
            dbg[nm] = nc.dram_tensor(nm, shp, dt_,
                                     kind="ExternalOutput").ap()

    xt = nc.dram_tensor("xt", [DIM, NTOK], DT, kind="ExternalInput").ap()
    wq = nc.dram_tensor("wq", [DIM, LDIM], DT, kind="ExternalInput").ap()
    wk = nc.dram_tensor("wk", [DIM, LDIM], DT, kind="ExternalInput").ap()
    wv = nc.dram_tensor("wv", [DIM, LDIM], DT, kind="ExternalInput").ap()
    wo = nc.dram_tensor("wo", [LDIM, DIM], DT, kind="ExternalInput").ap()
    qb = nc.dram_tensor("qb", [LDIM, 1], F32, kind="ExternalInput").ap()
    kb = nc.dram_tensor("kb", [LDIM, 1], F32, kind="ExternalInput").ap()
    vb = nc.dram_tensor("vb", [LDIM, 1], F32, kind="ExternalInput").ap()
    maskd = nc.dram_tensor("maskd", [128, BS * NKT], F32,
                           kind="ExternalInput").ap()
    out = nc.dram_tensor("out", [NTOK, DIM], F32, kind="ExternalOutput").ap()

    with tile.TileContext(nc) as tc, ExitStack() as ctx:
        singles = ctx.enter_context(tc.tile_pool(name="singles", bufs=1))
        evict = ctx.enter_context(tc.tile_pool(name="evict", bufs=2))
        stp = ctx.enter_context(tc.tile_pool(name="stp", bufs=3))
        srp = ctx.enter_context(tc.tile_pool(name="srp", bufs=2))
        work = ctx.enter_context(
            tc.tile_pool(name="work", bufs=2, space="PSUM"))
        accp = ctx.enter_context(
            tc.tile_pool(name="accp", bufs=2, space="PSUM"))

        # --- resident SBUF tensors ---
        wq_sb = singles.tile([128, NKCH, LDIM], DT, tag="wq")
        wk_sb = singles.tile([128, NKCH, LDIM], DT, tag="wk")
        wv_sb = singles.tile([128, NKCH, LDIM], DT, tag="wv")
        wo_sb = singles.tile([LDIM, DIM], DT, tag="wo")
        qb_sb = singles.tile([LDIM, 1], F32, tag="qb")
        kb_sb = singles.tile([LDIM, 1], F32, tag="kb")
        vb_sb = singles.tile([LDIM, 1], F32, tag="vb")
        mask_sb = singles.tile([128, BS * NKT], F32, tag="mask")
        onesA_sb = singles.tile([1, 128], DT, tag="onesA")
        onesB_sb = singles.tile([1, 128], DT, tag="onesB")
        qt_sb = singles.tile([128, NTOK], DT, tag="qt")
        kt_sb = singles.tile([128, NTOK], DT, tag="kt")
        vt_sb = singles.tile([128, NTOK], DT, tag="vt")
        ct_sb = singles.tile([128, NTOK], DT, tag="ct")
        rc_sb = singles.tile([128, QTW], F32, tag="rc")
        # V tiles per (batch, kt, head): [128 kpos, 64 dims | ones col]
        v65_sb = singles.tile([128, BS, 2 * NKT, 128], DT, tag="v65")
        # full x^T resident in SBUF, loaded one 512-token group per DMA
        xt_sb = singles.tile([128, NKCH, NTOK], DT, tag="xts")
        xt_r = xt.rearrange("(c p) t -> p c t", p=128)

        for w_sb, w_dram in ((wq_sb, wq), (wk_sb, wk), (wv_sb, wv)):
            nc.sync.dma_start(
                w_sb[:], w_dram.rearrange("(c p) m -> p c m", p=128))
        nc.sync.dma_start(wo_sb[:], wo[:])
        nc.sync.dma_start(qb_sb[:], qb[:])
        nc.sync.dma_start(kb_sb[:], kb[:])
        nc.sync.dma_start(vb_sb[:], vb[:])
        nc.sync.dma_start(mask_sb[:], maskd[:])
        nc.vector.memset(v65_sb[:], 1.0)
        nc.vector.memset(onesA_sb[:], 0.0)
        nc.vector.memset(onesA_sb[0:1, 0:64], 1.0)
        nc.vector.memset(onesB_sb[:], 0.0)
        nc.vector.memset(onesB_sb[0:1, 64:128], 1.0)

        # --- phase 1: QKV projections, token-tile at a time ---
        nc.sync.dma_start(xt_sb[:, :, 0:512], xt_r[:, :, 0:512])
        for g in range(NTT):
            if g + 1 < NTT:
                gn = slice(512 * (g + 1), 512 * (g + 2))
                nc.sync.dma_start(xt_sb[:, :, gn], xt_r[:, :, gn])
            psqk = work.tile([128, 1024], F32, tag="work")
            psv = accp.tile([128, 1024], F32, tag="acc")
            for c in range(NKCH):
                xt_t = xt_sb[:, c, 512 * g:512 * (g + 1)]
                st_, sp_ = (c == 0), (c == NKCH - 1)
                nc.tensor.matmul(psqk[:, 0:512], wq_sb[:, c, :], xt_t,
                                 start=st_, stop=sp_)
                nc.tensor.matmul(psqk[:, 512:1024], wk_sb[:, c, :], xt_t,
                                 start=st_, stop=sp_)
                nc.tensor.matmul(psv[:, 0:512], wv_sb[:, c, :], xt_t,
                                 start=st_, stop=sp_)
            gs = slice(512 * g, 512 * (g + 1))
            nc.vector.tensor_scalar_add(qt_sb[:, gs], psqk[:, 0:512],
                                        qb_sb[:, 0:1])
            nc.vector.tensor_scalar_add(kt_sb[:, gs], psqk[:, 512:1024],
                                        kb_sb[:, 0:1])
            nc.vector.tensor_scalar_add(vt_sb[:, gs], psv[:, 0:512],
                                        vb_sb[:, 0:1])

            # transpose this group's V token block into PV stationary tiles
            # ([64 dims, 128 kpos] -> [128 kpos, 64]); issued from Sync so the
            # xt loads (Pool queue) keep flowing.
            b = g // (NTT // BS)
            for j in range(4):
                ktl = 4 * (g % 4) + j
                t0 = QLEN * b + 128 * ktl
                for h in range(2):
                    src = vt_sb[HEAD * h:HEAD * (h + 1), t0:t0 + 128]
                    dst = v65_sb[:, b, ktl * 2 + h, 0:64]
                    nc.sync.dma_start(dst, src, transpose=True)

        # --- phase 2: attention + output projection per (batch, qtile) ---
        for b in range(BS):
            for qt_i in range(NQT):
                qs = slice(QLEN * b + QTW * qt_i, QLEN * b + QTW * (qt_i + 1))
                cts = []
                for h in range(2):
                    hs = slice(HEAD * h, HEAD * (h + 1))
                    ct = accp.tile([128, QTW], F32, tag="acc")
                    prev = None

                    def emit_pv(kt_i, st_t, ct=ct, b=b, h=h):
                        vsl = v65_sb[:, b, kt_i * 2 + h, 0:65]
                        st0, sp0 = (kt_i == 0), (kt_i == NKT - 1)
                        for j2 in range(2):
                            nc.tensor.matmul(
                                ct[0:65, 512 * j2:512 * (j2 + 1)],
                                vsl, st_t[:, 512 * j2:512 * (j2 + 1)],
                                start=st0, stop=sp0, skip_group_check=True)

                    for kt_i in range(NKT):
                        ks = slice(QLEN * b + 128 * kt_i,
                                   QLEN * b + 128 * (kt_i + 1))
                        s_ps = work.tile([128, QTW], F32, tag="work")
                        for j2 in range(2):
                            qsub = slice(qs.start + 512 * j2,
                                         qs.start + 512 * (j2 + 1))
                            nc.tensor.matmul(
                                s_ps[:, 512 * j2:512 * (j2 + 1)],
                                kt_sb[hs, ks], qt_sb[hs, qsub],
                                start=True, stop=True)
                        st_t = stp.tile([128, QTW], DT, tag="st")
                        m_ap = mask_sb[:, b * NKT + kt_i:b * NKT + kt_i + 1]
                        nc.scalar.activation(
                            st_t[:], s_ps[:], mybir.ActivationFunctionType.Exp,
                            bias=m_ap)
                        if prev is not None:
                            emit_pv(*prev)
                        prev = (kt_i, st_t)
                    emit_pv(*prev)
                    cts.append(ct)

                # denominators -> reciprocal -> normalize ctx into ct_sb
                srow0 = srp.tile([1, QTW], DT, tag="srow")
                srow1 = srp.tile([1, QTW], DT, tag="srow")
                nc.vector.tensor_copy(srow0[:], cts[0][64:65, :])
                nc.vector.tensor_copy(srow1[:], cts[1][64:65, :])
                sums_bc = work.tile([128, QTW], F32, tag="work")
                for j2 in range(2):
                    js = slice(512 * j2, 512 * (j2 + 1))
                    nc.tensor.matmul(sums_bc[:, js], onesA_sb[:],
                                     srow0[:, js], start=True, stop=False,
                                     skip_group_check=True)
                    nc.tensor.matmul(sums_bc[:, js], onesB_sb[:],
                                     srow1[:, js], start=False, stop=True,
                                     skip_group_check=True)
                nc.vector.reciprocal_approx_fast(rc_sb[:], sums_bc[:])
                nc.vector.tensor_mul(ct_sb[0:64, qs], cts[0][0:64, :],
                                     rc_sb[0:64, :])
                nc.vector.tensor_mul(ct_sb[64:128, qs], cts[1][0:64, :],
                                     rc_sb[64:128, :])

                # output projection for this qtile's tokens
                for t in range(QTW // 128):
                    tok0 = qs.start + 128 * t
                    o_ps = work.tile([128, 1024], F32, tag="work")
                    lhs = ct_sb[:, tok0:tok0 + 128]
                    nc.tensor.matmul(o_ps[:, 0:512], lhs, wo_sb[:, 0:512],
                                     start=True, stop=True)
                    nc.tensor.matmul(o_ps[:, 512:1024], lhs,
                                     wo_sb[:, 512:1024], start=True, stop=True)
                    o_sb = evict.tile([128, 1024], F32, tag="osb")
                    nc.vector.tensor_copy(o_sb[:], o_ps[:])
                    nc.sync.dma_start(out[tok0:tok0 + 128, :], o_sb[:])

        if dump:
            nc.sync.dma_start(dbg["d_qt"][:], qt_sb[:])
            nc.sync.dma_start(dbg["d_kt"][:], kt_sb[:])
            nc.sync.dma_start(dbg["d_vt"][:], vt_sb[:])
            nc.sync.dma_start(
                dbg["d_v65"][:],
                v65_sb.rearrange("p a b c -> p (a b c)"))
            nc.sync.dma_start(dbg["d_ct"][:], ct_sb[:])
            nc.sync.dma_start(dbg["d_rc"][:], rc_sb[:])

    nc.compile()
    return nc


def shard_inputs(input, mask, q_w, q_b, k_w, k_b, v_w, v_b, o_w, o_b):
    x = np.asarray(input, np.float32)
    xt = np.ascontiguousarray(x.T).astype(NPDT)
    m = np.asarray(mask, np.float32).reshape(BS, NKT, 128)
    maskd = np.ascontiguousarray(m.transpose(2, 0, 1).reshape(128, BS * NKT))
    scale = 1.0 / math.sqrt(HEAD)
    in_maps = []
    for c in range(NCORES):
        L = slice(LDIM * c, LDIM * (c + 1))
        in_maps.append({
            "xt": xt,
            "wq": np.ascontiguousarray((q_w[L, :] * scale).T).astype(NPDT),
            "wk": np.ascontiguousarray(k_w[L, :].T).astype(NPDT),
            "wv": np.ascontiguousarray(v_w[L, :].T).astype(NPDT),
            "wo": np.ascontiguousarray(o_w[:, L].T).astype(NPDT),
            "qb": (q_b[L] * scale).astype(np.float32).reshape(LDIM, 1),
            "kb": k_b[L].astype(np.float32).reshape(LDIM, 1),
            "vb": v_b[L].astype(np.float32).reshape(LDIM, 1),
            "maskd": maskd,
        })
    return in_maps


def run(in_maps, **kw):
    if "nc" not in _cache:
        _cache["nc"] = build_program()
    return run_bass_kernel_spmd(_cache["nc"], in_maps,
                                core_ids=list(range(NCORES)), **kw)


def kernel(input, mask, q_w, q_b, k_w, k_b, v_w, v_b, o_w, o_b,
           bs=BS, qlen=QLEN):
    assert int(bs) == BS and int(qlen) == QLEN
    in_maps = shard_inputs(np.asarray(input), np.asarray(mask),
                           np.asarray(q_w), np.asarray(q_b),
                           np.asarray(k_w), np.asarray(k_b),
                           np.asarray(v_w), np.asarray(v_b),
                           np.asarray(o_w), np.asarray(o_b))
    res = run(in_maps)
    acc = np.zeros((NTOK, DIM), np.float32)
    for r in res.results:
        acc += r["out"]
    acc += np.asarray(o_b, np.float32)[None, :]
    return acc


# revision 21
# speedup vs baseline: 1.5484x; 1.2431x over previous
"""MultiHeadAttention TRN2 kernel: tensor-parallel over heads across 8 NeuronCores.

Problem (hardcoded): BS=2, QLEN=2048, DIM=1024, NHEADS=16, HEAD=64.
  q = split_heads(x @ q_w.T + q_b) / sqrt(64)
  s = q @ k.T + mask ; w = softmax(s) ; ctx = w @ v
  out = merge_heads(ctx) @ o_w.T + o_b

Sharding: core c computes heads {2c, 2c+1} (rows 128c:128c+128 of q/k/v weights,
cols 128c:128c+128 of o_w).  Each core emits a full-shape partial of the output
projection; the host sums the 8 partials and adds o_b (row-parallel gather).

Device layout (all "T" tensors are [feature, token] so the contraction dim of
every matmul sits on SBUF partitions):
  QT/KT/VT [128, 4096]  = projections for 2 local heads (partitions = head dims)
  scores^T tiles [128 kpos, q]; exp() on ScalarE with the attention mask fed
  through the activation bias port (mask varies along k = partitions).
  PV stationary operands are V tiles [128 kpos, 65] with an extra all-ones
  column, so each PV accumulation also produces the softmax denominators for
  free (they land on one extra PSUM partition).  Head 0 uses [V|1] -> psum
  partitions 0..64 (sums at 64); head 1 uses [1|V] -> psum partitions 63..127
  (sums at 63), which leaves both heads' context dims aligned with the
  [2*64, q] layout the output projection wants.
  Denominators: copy the two sums rows to SBUF (bf16), broadcast to all 128
  partitions with a tiny [2,128]-stationary matmul, reciprocal_approx_fast on
  DVE, then two elementwise multiplies produce normalized ctx^T in SBUF.

Scheduling notes:
  - The attention inner loop is software-pipelined: scores(kt+1) runs on the
    PE while ScalarE exps kt and PV(kt-1) trails, keeping the PE continuously
    busy (it only reaches its top p-state after ~3us of continuous work).
  - All x input loads issue from the Pool queue and V transposes from Sync,
    spread one group at a time, so neither starves the other.
  - PSUM budget (8 banks): work pool 2x[128,1024]f32 + acc pool 2x[128,1024]f32.
"""

import sys

if "/opt/trn_rl_repo" not in sys.path:
    sys.path.insert(0, "/opt/trn_rl_repo")

import math
from contextlib import ExitStack

import ml_dtypes
import numpy as np

import concourse.bass as bass
import concourse.tile as tile
from concourse import bacc, mybir
from concourse.bass_utils import run_bass_kernel_spmd


# ---- problem constants ----
BS, QLEN, DIM, NHEADS = 2, 2048, 1024, 16
HEAD = DIM // NHEADS            # 64
NTOK = BS * QLEN                # 4096
NCORES = 8
HPC = NHEADS // NCORES          # 2 heads per core
LDIM = HPC * HEAD               # 128 local dims per core
NKCH = DIM // 128               # 8 contraction chunks for projections
NTT = NTOK // 512               # 8 token tiles of 512 for projections
NKT = QLEN // 128               # 16 key tiles per batch
QTW = 1024                      # query tile width for attention
NQT = QLEN // QTW               # 2 query tiles per batch

DT = mybir.dt.bfloat16          # matmul compute dtype
NPDT = ml_dtypes.bfloat16
F32 = mybir.dt.float32

_cache = {}


def build_program(dump=False):
    """Build + compile the single-core SPMD Bass program."""
    nc = bacc.Bacc("TRN2", target_bir_lowering=False, debug=False,
                   num_devices=NCORES)
    dbg = {}
    if dump:
        for nm, shp, dt_ in (("d_qt", [128, NTOK], DT),
                             ("d_kt", [128, NTOK], DT),
                             ("d_vt", [128, NTOK], DT),
                             ("d_v65", [128, BS * 2 * NKT * 128], DT),
                             ("d_ct", [128, NTOK], DT),
                             ("d_rc", [128, QTW], F32)):
            dbg[nm] = nc.dram_tensor(nm, shp, dt_,
                                     kind="ExternalOutput").ap()

    xt = nc.dram_tensor("xt", [DIM, NTOK], DT, kind="ExternalInput").ap()
    wq = nc.dram_tensor("wq", [DIM, LDIM], DT, kind="ExternalInput").ap()
    wk = nc.dram_tensor("wk", [DIM, LDIM], DT, kind="ExternalInput").ap()
    wv = nc.dram_tensor("wv", [DIM, LDIM], DT, kind="ExternalInput").ap()
    wo = nc.dram_tensor("wo", [LDIM, DIM], DT, kind="ExternalInput").ap()
    qb = nc.dram_tensor("qb", [LDIM, 1], F32, kind="ExternalInput").ap()
    kb = nc.dram_tensor("kb", [LDIM, 1], F32, kind="ExternalInput").ap()
    vb = nc.dram_tensor("vb", [LDIM, 1], F32, kind="ExternalInput").ap()
    maskd = nc.dram_tensor("maskd", [128, BS * NKT], F32,
                           kind="ExternalInput").ap()
    out = nc.dram_tensor("out", [NTOK, DIM], F32, kind="ExternalOutput").ap()

    with tile.TileContext(nc) as tc, ExitStack() as ctx:
        singles = ctx.enter_context(tc.tile_pool(name="singles", bufs=1))
        evict = ctx.enter_context(tc.tile_pool(name="evict", bufs=2))
        stp = ctx.enter_context(tc.tile_pool(name="stp", bufs=4))
        srp = ctx.enter_context(tc.tile_pool(name="srp", bufs=2))
        work = ctx.enter_context(
            tc.tile_pool(name="work", bufs=2, space="PSUM"))
        accp = ctx.enter_context(
            tc.tile_pool(name="accp", bufs=2, space="PSUM"))

        # --- resident SBUF tensors ---
        wq_sb = singles.tile([128, NKCH, LDIM], DT, tag="wq")
        wk_sb = singles.tile([128, NKCH, LDIM], DT, tag="wk")
        wv_sb = singles.tile([128, NKCH, LDIM], DT, tag="wv")
        wo_sb = singles.tile([LDIM, DIM], DT, tag="wo")
        qb_sb = singles.tile([LDIM, 1], F32, tag="qb")
        kb_sb = singles.tile([LDIM, 1], F32, tag="kb")
        vb_sb = singles.tile([LDIM, 1], F32, tag="vb")
        mask_sb = singles.tile([128, BS * NKT], F32, tag="mask")
        onesA_sb = singles.tile([1, 128], DT, tag="onesA")
        onesB_sb = singles.tile([1, 128], DT, tag="onesB")
        qt_sb = singles.tile([128, NTOK], DT, tag="qt")
        kt_sb = singles.tile([128, NTOK], DT, tag="kt")
        vt_sb = singles.tile([128, NTOK], DT, tag="vt")
        ct_sb = singles.tile([128, NTOK], DT, tag="ct")
        rc_sb = singles.tile([128, QTW], F32, tag="rc")
        # V tiles per (batch, kt, head): [128 kpos, 64 dims | ones col]
        v65_sb = singles.tile([128, BS, 2 * NKT, 128], DT, tag="v65")
        # full x^T resident in SBUF, loaded one 512-token group per DMA
        xt_sb = singles.tile([128, NKCH, NTOK], DT, tag="xts")
        xt_r = xt.rearrange("(c p) t -> p c t", p=128)

        for w_sb, w_dram in ((wq_sb, wq), (wk_sb, wk), (wv_sb, wv)):
            nc.sync.dma_start(
                w_sb[:], w_dram.rearrange("(c p) m -> p c m", p=128))
        nc.sync.dma_start(wo_sb[:], wo[:])
        nc.sync.dma_start(qb_sb[:], qb[:])
        nc.sync.dma_start(kb_sb[:], kb[:])
        nc.sync.dma_start(vb_sb[:], vb[:])
        nc.sync.dma_start(mask_sb[:], maskd[:])
        nc.vector.memset(v65_sb[:], 1.0)
        nc.vector.memset(onesA_sb[:], 0.0)
        nc.vector.memset(onesA_sb[0:1, 0:64], 1.0)
        nc.vector.memset(onesB_sb[:], 0.0)
        nc.vector.memset(onesB_sb[0:1, 64:128], 1.0)

        # --- phase 1: QKV projections, token-tile at a time ---
        nc.sync.dma_start(xt_sb[:, :, 0:512], xt_r[:, :, 0:512])
        for g in range(NTT):
            if g + 1 < NTT:
                gn = slice(512 * (g + 1), 512 * (g + 2))
                nc.sync.dma_start(xt_sb[:, :, gn], xt_r[:, :, gn])
            psqk = work.tile([128, 1024], F32, tag="work")
            psv = accp.tile([128, 1024], F32, tag="acc")
            for c in range(NKCH):
                xt_t = xt_sb[:, c, 512 * g:512 * (g + 1)]
                st_, sp_ = (c == 0), (c == NKCH - 1)
                nc.tensor.matmul(psqk[:, 0:512], wq_sb[:, c, :], xt_t,
                                 start=st_, stop=sp_)
                nc.tensor.matmul(psqk[:, 512:1024], wk_sb[:, c, :], xt_t,
                                 start=st_, stop=sp_)
                nc.tensor.matmul(psv[:, 0:512], wv_sb[:, c, :], xt_t,
                                 start=st_, stop=sp_)
            gs = slice(512 * g, 512 * (g + 1))
            nc.vector.tensor_scalar_add(qt_sb[:, gs], psqk[:, 0:512],
                                        qb_sb[:, 0:1])
            nc.vector.tensor_scalar_add(kt_sb[:, gs], psqk[:, 512:1024],
                                        kb_sb[:, 0:1])
            nc.vector.tensor_scalar_add(vt_sb[:, gs], psv[:, 0:512],
                                        vb_sb[:, 0:1])

            # transpose batch-0 V token blocks into PV stationary tiles
            # ([64 dims, 128 kpos] -> [128 kpos, 64]).  b0's transposes issue
            # from the ScalarE queue (idle until attention starts); b1's are
            # deferred into the first attention qtile's loop on Sync.
            b = g // (NTT // BS)
            if b == 0:
                for j in range(4):
                    ktl = 4 * (g % 4) + j
                    t0 = QLEN * b + 128 * ktl
                    for h in range(2):
                        src = vt_sb[HEAD * h:HEAD * (h + 1), t0:t0 + 128]
                        dst = v65_sb[:, b, ktl * 2 + h, 0:64]
                        nc.scalar.dma_start(dst, src, transpose=True)

        # --- phase 2: attention + output projection per (batch, qtile) ---
        # Deferred-work queue: items emitted one per kt unit inside the next
        # qtile's loop so the divide / output projection / DMA issue work of
        # qtile i overlaps qtile i+1's scores+PV stream.
        from collections import deque
        pending = deque()

        def defer_transpose(b, ktl, h):
            def go():
                t0 = QLEN * b + 128 * ktl
                src = vt_sb[HEAD * h:HEAD * (h + 1), t0:t0 + 128]
                dst = v65_sb[:, b, ktl * 2 + h, 0:64]
                nc.sync.dma_start(dst, src, transpose=True)
            return go

        for ktl in range(NKT):
            for h in range(2):
                pending.append(defer_transpose(1, ktl, h))

        def emit_divide(cts, qs):
            """Denominators -> reciprocal -> normalized ctx in ct_sb."""
            srow0 = srp.tile([1, QTW], DT, tag="srow", name="srow0")
            srow1 = srp.tile([1, QTW], DT, tag="srow", name="srow1")
            nc.vector.tensor_copy(srow0[:], cts[0][64:65, :])
            nc.vector.tensor_copy(srow1[:], cts[1][64:65, :])
            sums_bc = work.tile([128, QTW], F32, tag="work", name="sums_bc")
            for j2 in range(2):
                js = slice(512 * j2, 512 * (j2 + 1))
                nc.tensor.matmul(sums_bc[:, js], onesA_sb[:],
                                 srow0[:, js], start=True, stop=False,
                                 skip_group_check=True)
                nc.tensor.matmul(sums_bc[:, js], onesB_sb[:],
                                 srow1[:, js], start=False, stop=True,
                                 skip_group_check=True)
            nc.vector.reciprocal_approx_fast(rc_sb[:], sums_bc[:])
            nc.vector.tensor_mul(ct_sb[0:64, qs], cts[0][0:64, :],
                                 rc_sb[0:64, :])
            nc.vector.tensor_mul(ct_sb[64:128, qs], cts[1][0:64, :],
                                 rc_sb[64:128, :])

        def defer_outproj(qs, t):
            def go():
                tok0 = qs.start + 128 * t
                o_ps = work.tile([128, 1024], F32, tag="work", name="o_ps")
                lhs = ct_sb[:, tok0:tok0 + 128]
                nc.tensor.matmul(o_ps[:, 0:512], lhs, wo_sb[:, 0:512],
                                 start=True, stop=True)
                nc.tensor.matmul(o_ps[:, 512:1024], lhs,
                                 wo_sb[:, 512:1024], start=True, stop=True)
                o_sb = evict.tile([128, 1024], F32, tag="osb", name="o_sb")
                nc.vector.tensor_copy(o_sb[:], o_ps[:])
                nc.sync.dma_start(out[tok0:tok0 + 128, :], o_sb[:])
            return go

        prev_q = None  # (cts, qs) of the previous qtile, divide not yet done
        for b in range(BS):
            for qt_i in range(NQT):
                qs = slice(QLEN * b + QTW * qt_i, QLEN * b + QTW * (qt_i + 1))
                cts = []
                for h in range(2):
                    hs = slice(HEAD * h, HEAD * (h + 1))
                    ct = accp.tile([128, QTW], F32, tag="acc", name="ct")

                    def emit_pv(kt_i, st_t, ct=ct, b=b, h=h):
                        vsl = v65_sb[:, b, kt_i * 2 + h, 0:65]
                        st0, sp0 = (kt_i == 0), (kt_i == NKT - 1)
                        for j2 in range(2):
                            nc.tensor.matmul(
                                ct[0:65, 512 * j2:512 * (j2 + 1)],
                                vsl, st_t[:, 512 * j2:512 * (j2 + 1)],
                                start=st0, stop=sp0, skip_group_check=True)

                    lagged = []
                    for kt_i in range(NKT):
                        ks = slice(QLEN * b + 128 * kt_i,
                                   QLEN * b + 128 * (kt_i + 1))
                        s_ps = work.tile([128, QTW], F32, tag="work",
                                         name="s_ps")
                        for j2 in range(2):
                            qsub = slice(qs.start + 512 * j2,
                                         qs.start + 512 * (j2 + 1))
                            nc.tensor.matmul(
                                s_ps[:, 512 * j2:512 * (j2 + 1)],
                                kt_sb[hs, ks], qt_sb[hs, qsub],
                                start=True, stop=True)
                        st_t = stp.tile([128, QTW], DT, tag="st", name="st_t")
                        m_ap = mask_sb[:, b * NKT + kt_i:b * NKT + kt_i + 1]
                        nc.scalar.activation(
                            st_t[:], s_ps[:], mybir.ActivationFunctionType.Exp,
                            bias=m_ap)
                        lagged.append((kt_i, st_t))
                        # previous qtile's divide goes early so its outproj
                        # (and this qtile's first PV via the accp ring) unblock
                        if h == 0 and kt_i == 1 and prev_q is not None:
                            emit_divide(*prev_q)
                            for t in range(QTW // 128):
                                pending.append(defer_outproj(prev_q[1], t))
                            prev_q = None
                        if len(lagged) > 2:
                            emit_pv(*lagged.pop(0))
                        if kt_i >= 3 and pending:
                            pending.popleft()()
                    for item in lagged:
                        emit_pv(*item)
                    cts.append(ct)
                prev_q = (cts, qs)

        # final qtile: drain
        emit_divide(*prev_q)
        for t in range(QTW // 128):
            defer_outproj(prev_q[1], t)()
        while pending:
            pending.popleft()()

        if dump:
            nc.sync.dma_start(dbg["d_qt"][:], qt_sb[:])
            nc.sync.dma_start(dbg["d_kt"][:], kt_sb[:])
            nc.sync.dma_start(dbg["d_vt"][:], vt_sb[:])
            nc.sync.dma_start(
                dbg["d_v65"][:],
                v65_sb.rearrange("p a b c -> p (a b c)"))
            nc.sync.dma_start(dbg["d_ct"][:], ct_sb[:])
            nc.sync.dma_start(dbg["d_rc"][:], rc_sb[:])

    nc.compile()
    return nc


def shard_inputs(input, mask, q_w, q_b, k_w, k_b, v_w, v_b, o_w, o_b):
    x = np.asarray(input, np.float32)
    xt = np.ascontiguousarray(x.T).astype(NPDT)
    m = np.asarray(mask, np.float32).reshape(BS, NKT, 128)
    maskd = np.ascontiguousarray(m.transpose(2, 0, 1).reshape(128, BS * NKT))
    scale = 1.0 / math.sqrt(HEAD)
    in_maps = []
    for c in range(NCORES):
        L = slice(LDIM * c, LDIM * (c + 1))
        in_maps.append({
            "xt": xt,
            "wq": np.ascontiguousarray((q_w[L, :] * scale).T).astype(NPDT),
            "wk": np.ascontiguousarray(k_w[L, :].T).astype(NPDT),
            "wv": np.ascontiguousarray(v_w[L, :].T).astype(NPDT),
            "wo": np.ascontiguousarray(o_w[:, L].T).astype(NPDT),
            "qb": (q_b[L] * scale).astype(np.float32).reshape(LDIM, 1),
            "kb": k_b[L].astype(np.float32).reshape(LDIM, 1),
            "vb": v_b[L].astype(np.float32).reshape(LDIM, 1),
            "maskd": maskd,
        })
    return in_maps


def run(in_maps, **kw):
    if "nc" not in _cache:
        _cache["nc"] = build_program()
    return run_bass_kernel_spmd(_cache["nc"], in_maps,
                                core_ids=list(range(NCORES)), **kw)


def kernel(input, mask, q_w, q_b, k_w, k_b, v_w, v_b, o_w, o_b,
           bs=BS, qlen=QLEN):
    assert int(bs) == BS and int(qlen) == QLEN
    in_maps = shard_inputs(np.asarray(input), np.asarray(mask),
                           np.asarray(q_w), np.asarray(q_b),
                           np.asarray(k_w), np.asarray(k_b),
                           np.asarray(v_w), np.asarray(v_b),
                           np.asarray(o_w), np.asarray(o_b))
    res = run(in_maps)
    acc = np.zeros((NTOK, DIM), np.float32)
    for r in res.results:
        acc += r["out"]
    acc += np.asarray(o_b, np.float32)[None, :]
    return acc


# revision 32
# speedup vs baseline: 1.6643x; 1.0748x over previous
"""MultiHeadAttention TRN2 kernel: tensor-parallel over heads across 8 NeuronCores.

Problem (hardcoded): BS=2, QLEN=2048, DIM=1024, NHEADS=16, HEAD=64.
  q = split_heads(x @ q_w.T + q_b) / sqrt(64)
  s = q @ k.T + mask ; w = softmax(s) ; ctx = w @ v
  out = merge_heads(ctx) @ o_w.T + o_b

Sharding: core c computes heads {2c, 2c+1} (rows 128c:128c+128 of q/k/v weights,
cols 128c:128c+128 of o_w).  Each core emits a full-shape partial of the output
projection; the host sums the 8 partials and adds o_b (row-parallel gather).

Device layout (all "T" tensors are [feature, token] so the contraction dim of
every matmul sits on SBUF partitions):
  QT/KT/VT [128, 4096]  = projections for 2 local heads (partitions = head dims)
  scores^T tiles [128 kpos, q]; exp() on ScalarE with the attention mask fed
  through the activation bias port (mask varies along k = partitions).
  PV stationary operands are V tiles [128 kpos, 65] with an extra all-ones
  column, so each PV accumulation also produces the softmax denominators for
  free (they land on one extra PSUM partition).  Head 0 uses [V|1] -> psum
  partitions 0..64 (sums at 64); head 1 uses [1|V] -> psum partitions 63..127
  (sums at 63), which leaves both heads' context dims aligned with the
  [2*64, q] layout the output projection wants.
  Denominators: copy the two sums rows to SBUF (bf16), broadcast to all 128
  partitions with a tiny [2,128]-stationary matmul, reciprocal_approx_fast on
  DVE, then two elementwise multiplies produce normalized ctx^T in SBUF.

Scheduling notes:
  - The attention inner loop is software-pipelined: scores(kt+1) runs on the
    PE while ScalarE exps kt and PV(kt-1) trails, keeping the PE continuously
    busy (it only reaches its top p-state after ~3us of continuous work).
  - All x input loads issue from the Pool queue and V transposes from Sync,
    spread one group at a time, so neither starves the other.
  - PSUM budget (8 banks): work pool 2x[128,1024]f32 + acc pool 2x[128,1024]f32.
"""

import sys

if "/opt/trn_rl_repo" not in sys.path:
    sys.path.insert(0, "/opt/trn_rl_repo")

import math
from contextlib import ExitStack

import ml_dtypes
import numpy as np

import concourse.bass as bass
import concourse.tile as tile
from concourse import bacc, mybir
from concourse.bass_utils import run_bass_kernel_spmd


# ---- problem constants ----
BS, QLEN, DIM, NHEADS = 2, 2048, 1024, 16
HEAD = DIM // NHEADS            # 64
NTOK = BS * QLEN                # 4096
NCORES = 8
HPC = NHEADS // NCORES          # 2 heads per core
LDIM = HPC * HEAD               # 128 local dims per core
NKCH = DIM // 128               # 8 contraction chunks for projections
NTT = NTOK // 512               # 8 token tiles of 512 for projections
NKT = QLEN // 128               # 16 key tiles per batch
QTW = 1024                      # query tile width for attention
NQT = QLEN // QTW               # 2 query tiles per batch

DT = mybir.dt.bfloat16          # matmul compute dtype
NPDT = ml_dtypes.bfloat16
F32 = mybir.dt.float32

_cache = {}


def build_program(dump=False):
    """Build + compile the single-core SPMD Bass program."""
    nc = bacc.Bacc("TRN2", target_bir_lowering=False, debug=False,
                   num_devices=NCORES)
    dbg = {}
    if dump:
        for nm, shp, dt_ in (("d_qt", [128, NTOK], DT),
                             ("d_kt", [128, NTOK], DT),
                             ("d_vt", [128, NTOK], DT),
                             ("d_v65", [128, BS * 2 * NKT * 128], DT),
                             ("d_ct", [128, NTOK], DT),
                             ("d_rc", [128, QTW], F32)):
            dbg[nm] = nc.dram_tensor(nm, shp, dt_,
                                     kind="ExternalOutput").ap()

    xt = nc.dram_tensor("xt", [DIM, NTOK], DT, kind="ExternalInput").ap()
    wq = nc.dram_tensor("wq", [DIM, LDIM], DT, kind="ExternalInput").ap()
    wk = nc.dram_tensor("wk", [DIM, LDIM], DT, kind="ExternalInput").ap()
    wv = nc.dram_tensor("wv", [DIM, LDIM], DT, kind="ExternalInput").ap()
    wo = nc.dram_tensor("wo", [LDIM, DIM], DT, kind="ExternalInput").ap()
    qb = nc.dram_tensor("qb", [LDIM, 1], F32, kind="ExternalInput").ap()
    kb = nc.dram_tensor("kb", [LDIM, 1], F32, kind="ExternalInput").ap()
    vb = nc.dram_tensor("vb", [LDIM, 1], F32, kind="ExternalInput").ap()
    maskd = nc.dram_tensor("maskd", [128, BS * NKT], F32,
                           kind="ExternalInput").ap()
    out = nc.dram_tensor("out", [NTOK, DIM], DT, kind="ExternalOutput").ap()

    with tile.TileContext(nc) as tc, ExitStack() as ctx:
        singles = ctx.enter_context(tc.tile_pool(name="singles", bufs=1))
        evict = ctx.enter_context(tc.tile_pool(name="evict", bufs=2))
        stp = ctx.enter_context(tc.tile_pool(name="stp", bufs=5))
        srp = ctx.enter_context(tc.tile_pool(name="srp", bufs=2))
        work = ctx.enter_context(
            tc.tile_pool(name="work", bufs=2, space="PSUM"))
        accp = ctx.enter_context(
            tc.tile_pool(name="accp", bufs=2, space="PSUM"))

        # --- resident SBUF tensors ---
        wq_sb = singles.tile([128, NKCH, LDIM], DT, tag="wq")
        wk_sb = singles.tile([128, NKCH, LDIM], DT, tag="wk")
        wv_sb = singles.tile([128, NKCH, LDIM], DT, tag="wv")
        wo_sb = singles.tile([LDIM, DIM], DT, tag="wo")
        qb_sb = singles.tile([LDIM, 1], F32, tag="qb")
        kb_sb = singles.tile([LDIM, 1], F32, tag="kb")
        vb_sb = singles.tile([LDIM, 1], F32, tag="vb")
        mask_sb = singles.tile([128, BS * NKT], F32, tag="mask")
        onesA_sb = singles.tile([1, 128], DT, tag="onesA")
        onesB_sb = singles.tile([1, 128], DT, tag="onesB")
        qt_sb = singles.tile([128, NTOK], DT, tag="qt")
        kt_sb = singles.tile([128, NTOK], DT, tag="kt")
        vt_sb = singles.tile([128, NTOK], DT, tag="vt")
        ct_sb = singles.tile([128, NTOK], DT, tag="ct")
        rc_sb = singles.tile([128, QTW], F32, tag="rc")
        # V tiles per (batch, kt, head): [128 kpos, 64 dims | ones col]
        v65_sb = singles.tile([128, BS, 2 * NKT, 128], DT, tag="v65")
        # full x^T resident in SBUF, loaded one 512-token group per DMA
        xt_sb = singles.tile([128, NKCH, NTOK], DT, tag="xts")
        xt_r = xt.rearrange("(c p) t -> p c t", p=128)

        # load order: first QKV weights + the first x group (the critical
        # path to the first matmul), then everything needed later.
        for w_sb, w_dram in ((wq_sb, wq), (wk_sb, wk), (wv_sb, wv)):
            nc.sync.dma_start(
                w_sb[:], w_dram.rearrange("(c p) m -> p c m", p=128))
        nc.sync.dma_start(qb_sb[:], qb[:])
        nc.sync.dma_start(kb_sb[:], kb[:])
        nc.sync.dma_start(vb_sb[:], vb[:])
        nc.scalar.dma_start(mask_sb[:], maskd[:])
        nc.scalar.dma_start(wo_sb[:], wo[:])
        nc.vector.memset(v65_sb[:, :, :, 64:65], 1.0)
        nc.vector.memset(onesA_sb[:], 0.0)
        nc.vector.memset(onesA_sb[0:1, 0:64], 1.0)
        nc.vector.memset(onesB_sb[:], 0.0)
        nc.vector.memset(onesB_sb[0:1, 64:128], 1.0)

        # --- phase 1: QKV projections, token-tile at a time ---
        nc.sync.dma_start(xt_sb[:, :, 0:512], xt_r[:, :, 0:512])
        for g in range(NTT):
            if g + 1 < NTT:
                gn = slice(512 * (g + 1), 512 * (g + 2))
                nc.sync.dma_start(xt_sb[:, :, gn], xt_r[:, :, gn])
            psqk = work.tile([128, 1024], F32, tag="work")
            psv = accp.tile([128, 1024], F32, tag="acc")
            for c in range(NKCH):
                xt_t = xt_sb[:, c, 512 * g:512 * (g + 1)]
                st_, sp_ = (c == 0), (c == NKCH - 1)
                nc.tensor.matmul(psqk[:, 0:512], wq_sb[:, c, :], xt_t,
                                 start=st_, stop=sp_)
                nc.tensor.matmul(psqk[:, 512:1024], wk_sb[:, c, :], xt_t,
                                 start=st_, stop=sp_)
                nc.tensor.matmul(psv[:, 0:512], wv_sb[:, c, :], xt_t,
                                 start=st_, stop=sp_)
            gs = slice(512 * g, 512 * (g + 1))
            nc.vector.tensor_scalar_add(qt_sb[:, gs], psqk[:, 0:512],
                                        qb_sb[:, 0:1])
            nc.vector.tensor_scalar_add(kt_sb[:, gs], psqk[:, 512:1024],
                                        kb_sb[:, 0:1])
            nc.vector.tensor_scalar_add(vt_sb[:, gs], psv[:, 0:512],
                                        vb_sb[:, 0:1])

            # transpose batch-0 V token blocks into PV stationary tiles
            # ([64 dims, 128 kpos] -> [128 kpos, 64]).  b0's transposes issue
            # from the ScalarE queue (idle until attention starts); b1's are
            # deferred into the first attention qtile's loop on Sync.
            b = g // (NTT // BS)
            if b == 0:
                for j in range(4):
                    ktl = 4 * (g % 4) + j
                    t0 = QLEN * b + 128 * ktl
                    for h in range(2):
                        src = vt_sb[HEAD * h:HEAD * (h + 1), t0:t0 + 128]
                        dst = v65_sb[:, b, ktl * 2 + h, 0:64]
                        eng = nc.scalar if (j + h) % 2 else nc.sync
                        eng.dma_start(dst, src, transpose=True)

        # --- phase 2: attention + output projection per (batch, qtile) ---
        # Deferred-work queue: items emitted one per kt unit inside the next
        # qtile's loop so the divide / output projection / DMA issue work of
        # qtile i overlaps qtile i+1's scores+PV stream.
        from collections import deque
        pend_dma = deque()   # DMA-issue-only items: pop one per kt unit
        pend_pe = deque()    # PE-work items (outproj): pop every other unit

        def defer_transpose(b, ktl, h):
            def go():
                t0 = QLEN * b + 128 * ktl
                src = vt_sb[HEAD * h:HEAD * (h + 1), t0:t0 + 128]
                dst = v65_sb[:, b, ktl * 2 + h, 0:64]
                nc.sync.dma_start(dst, src, transpose=True)
            return go

        for ktl in range(NKT):
            for h in range(2):
                pend_dma.append(defer_transpose(1, ktl, h))

        def emit_divide(cts, qs):
            """Denominators -> reciprocal -> normalized ctx in ct_sb."""
            srow0 = srp.tile([1, QTW], DT, tag="srow", name="srow0")
            srow1 = srp.tile([1, QTW], DT, tag="srow", name="srow1")
            nc.vector.tensor_copy(srow0[:], cts[0][64:65, :])
            nc.vector.tensor_copy(srow1[:], cts[1][64:65, :])
            sums_bc = work.tile([128, QTW], F32, tag="work", name="sums_bc")
            for j2 in range(2):
                js = slice(512 * j2, 512 * (j2 + 1))
                nc.tensor.matmul(sums_bc[:, js], onesA_sb[:],
                                 srow0[:, js], start=True, stop=False,
                                 skip_group_check=True)
                nc.tensor.matmul(sums_bc[:, js], onesB_sb[:],
                                 srow1[:, js], start=False, stop=True,
                                 skip_group_check=True)
            nc.vector.reciprocal_approx_fast(rc_sb[:], sums_bc[:])
            nc.vector.tensor_mul(ct_sb[0:64, qs], cts[0][0:64, :],
                                 rc_sb[0:64, :])
            nc.vector.tensor_mul(ct_sb[64:128, qs], cts[1][0:64, :],
                                 rc_sb[64:128, :])

        def defer_outproj(qs, t):
            def go():
                tok0 = qs.start + 128 * t
                o_ps = work.tile([128, 1024], F32, tag="work", name="o_ps")
                lhs = ct_sb[:, tok0:tok0 + 128]
                nc.tensor.matmul(o_ps[:, 0:512], lhs, wo_sb[:, 0:512],
                                 start=True, stop=True)
                nc.tensor.matmul(o_ps[:, 512:1024], lhs,
                                 wo_sb[:, 512:1024], start=True, stop=True)
                o_sb = evict.tile([128, 1024], DT, tag="osb", name="o_sb")
                nc.vector.tensor_copy(o_sb[:], o_ps[:])
                nc.sync.dma_start(out[tok0:tok0 + 128, :], o_sb[:])
            return go

        prev_q = None  # (cts, qs) of the previous qtile, divide not yet done
        for b in range(BS):
            for qt_i in range(NQT):
                qs = slice(QLEN * b + QTW * qt_i, QLEN * b + QTW * (qt_i + 1))
                cts = []
                for h in range(2):
                    hs = slice(HEAD * h, HEAD * (h + 1))
                    ct = accp.tile([128, QTW], F32, tag="acc", name="ct")

                    def emit_pv(kt_i, st_t, ct=ct, b=b, h=h):
                        vsl = v65_sb[:, b, kt_i * 2 + h, 0:65]
                        st0, sp0 = (kt_i == 0), (kt_i == NKT - 1)
                        for j2 in range(2):
                            nc.tensor.matmul(
                                ct[0:65, 512 * j2:512 * (j2 + 1)],
                                vsl, st_t[:, 512 * j2:512 * (j2 + 1)],
                                start=st0, stop=sp0, skip_group_check=True)

                    lagged = []
                    for kt_i in range(NKT):
                        ks = slice(QLEN * b + 128 * kt_i,
                                   QLEN * b + 128 * (kt_i + 1))
                        s_ps = work.tile([128, QTW], F32, tag="work",
                                         name="s_ps")
                        for j2 in range(2):
                            qsub = slice(qs.start + 512 * j2,
                                         qs.start + 512 * (j2 + 1))
                            nc.tensor.matmul(
                                s_ps[:, 512 * j2:512 * (j2 + 1)],
                                kt_sb[hs, ks], qt_sb[hs, qsub],
                                start=True, stop=True)
                        st_t = stp.tile([128, QTW], DT, tag="st", name="st_t")
                        m_ap = mask_sb[:, b * NKT + kt_i:b * NKT + kt_i + 1]
                        nc.scalar.activation(
                            st_t[:], s_ps[:], mybir.ActivationFunctionType.Exp,
                            bias=m_ap)
                        lagged.append((kt_i, st_t))
                        # previous qtile's divide goes early so its outproj
                        # (and this qtile's first PV via the accp ring) unblock
                        if h == 0 and kt_i == 1 and prev_q is not None:
                            emit_divide(*prev_q)
                            for t in range(QTW // 128):
                                pend_pe.append(defer_outproj(prev_q[1], t))
                            prev_q = None
                        if len(lagged) > 3:
                            emit_pv(*lagged.pop(0))
                        if kt_i >= 2 and pend_dma:
                            pend_dma.popleft()()
                        if kt_i >= 3 and kt_i % 2 == 1 and pend_pe:
                            pend_pe.popleft()()
                    for item in lagged:
                        emit_pv(*item)
                    cts.append(ct)
                prev_q = (cts, qs)

        # final qtile: drain
        emit_divide(*prev_q)
        for t in range(QTW // 128):
            defer_outproj(prev_q[1], t)()
        while pend_dma:
            pend_dma.popleft()()
        while pend_pe:
            pend_pe.popleft()()

        if dump:
            nc.sync.dma_start(dbg["d_qt"][:], qt_sb[:])
            nc.sync.dma_start(dbg["d_kt"][:], kt_sb[:])
            nc.sync.dma_start(dbg["d_vt"][:], vt_sb[:])
            nc.sync.dma_start(
                dbg["d_v65"][:],
                v65_sb.rearrange("p a b c -> p (a b c)"))
            nc.sync.dma_start(dbg["d_ct"][:], ct_sb[:])
            nc.sync.dma_start(dbg["d_rc"][:], rc_sb[:])

    nc.compile()
    return nc


def shard_inputs(input, mask, q_w, q_b, k_w, k_b, v_w, v_b, o_w, o_b):
    x = np.asarray(input, np.float32)
    xt = np.ascontiguousarray(x.T).astype(NPDT)
    m = np.asarray(mask, np.float32).reshape(BS, NKT, 128)
    maskd = np.ascontiguousarray(m.transpose(2, 0, 1).reshape(128, BS * NKT))
    scale = 1.0 / math.sqrt(HEAD)
    in_maps = []
    for c in range(NCORES):
        L = slice(LDIM * c, LDIM * (c + 1))
        in_maps.append({
            "xt": xt,
            "wq": np.ascontiguousarray((q_w[L, :] * scale).T).astype(NPDT),
            "wk": np.ascontiguousarray(k_w[L, :].T).astype(NPDT),
            "wv": np.ascontiguousarray(v_w[L, :].T).astype(NPDT),
            "wo": np.ascontiguousarray(o_w[:, L].T).astype(NPDT),
            "qb": (q_b[L] * scale).astype(np.float32).reshape(LDIM, 1),
            "kb": k_b[L].astype(np.float32).reshape(LDIM, 1),
            "vb": v_b[L].astype(np.float32).reshape(LDIM, 1),
            "maskd": maskd,
        })
    return in_maps


def run(in_maps, **kw):
    if "nc" not in _cache:
        _cache["nc"] = build_program()
    return run_bass_kernel_spmd(_cache["nc"], in_maps,
                                core_ids=list(range(NCORES)), **kw)


def kernel(input, mask, q_w, q_b, k_w, k_b, v_w, v_b, o_w, o_b,
           bs=BS, qlen=QLEN):
    assert int(bs) == BS and int(qlen) == QLEN
    in_maps = shard_inputs(np.asarray(input), np.asarray(mask),
                           np.asarray(q_w), np.asarray(q_b),
                           np.asarray(k_w), np.asarray(k_b),
                           np.asarray(v_w), np.asarray(v_b),
                           np.asarray(o_w), np.asarray(o_b))
    res = run(in_maps)
    acc = np.zeros((NTOK, DIM), np.float32)
    for r in res.results:
        acc += np.asarray(r["out"], np.float32)
    acc += np.asarray(o_b, np.float32)[None, :]
    return acc
